# revision 1
# baseline (speedup 1.0000x reference)
"""Trainium2 Bass kernel v5 for nn_AaD_MAPU (retrieval kNN shortlist).

HW-legal fold pipeline (GPSIMD/Pool cannot touch PSUM or run TensorTensor;
DMA cannot read PSUM — so PSUM is drained by ScalarE and VectorE only, and
the DMA's spare bandwidth ships converted tiles to the host, which folds
them in numpy):

  - dp half-tiles [128, 2bc, W] fp32, 4 PSUM bufs (2 banks each)
  - path A : ScalarE copy fp32->bf16 (1038ns) + VectorE bf16 2x fold (593ns)
  - path A2: ScalarE copy fp32->bf16 (1038ns) + DMA to DRAM (728ns), host
             folds the raw bf16 tile
  - path B : VectorE folds straight from PSUM into accV (1192ns)
  - fnt rides as the first 512 columns of the fbt DRAM tensor

Host: max(accV, folded raws) -> 512-wide combs -> phase-max to 128-wide ->
global top-T -> exact fp32 re-rank of comb members (pos = e + 128k).

Tile widths: 256, 256, 512 x23, 256 (all starts multiples of 128).
"""

from contextlib import ExitStack

import numpy as np

import concourse.bass as bass
import concourse.tile as tile
from concourse import bacc, mybir
from concourse.bass_utils import run_bass_kernel_spmd

B, D, N, C, K = 512, 512, 100000, 64, 5
EPS = 1e-12
NCORES = 8
NSHARD = 12544
NPAD = NSHARD * NCORES
BCH = 4
FSCALE = 16.0
NEG = -3.0e38

WIDTHS = [256, 256] + [512] * 23 + [256]
N_WARMUP = 7
NA2 = 18   # number of A2 (DMA-out) halves
NB = 20    # number of B (VectorE-direct) halves

_F32 = mybir.dt.float32
_BF16 = mybir.dt.bfloat16
_FP8 = mybir.dt.float8e4

_cache = {}


def _assign_paths():
    """52 halves, Bresenham-spread quotas A=16, B=20, A2=16.

    Per-engine loads (fulls): Act = (A+A2) x1038 ~= 33us, V = A x593 +
    B x1192 ~= 33us, DMA = fbt stream + A2 x728 ~= 33us. A2 stays out of
    the first 4 halves (DMA prefetch headroom) and the last 2 (so the
    final acc DMA chain isn't queued behind raw-tile DMAs).
    """
    n = 2 * len(WIDTHS)
    quota = {"A": n - NB - NA2, "B": NB, "2": NA2}
    issued = {"A": 0, "B": 0, "2": 0}
    out = []
    for i in range(n):
        cand = [p for p in ("A", "B", "2") if issued[p] < quota[p]]
        if i < 4 or i >= n - 2:
            cand = [p for p in cand if p != "2"] or cand
        p = min(cand, key=lambda q: (issued[q] + 0.5) / quota[q])
        issued[p] += 1
        out.append(p)
    return out


def _build_module():
    nc = bacc.Bacc("TRN2", target_bir_lowering=False, debug=False,
                   num_devices=NCORES)
    # columns 0:512 = f_norm.T * FSCALE, columns 512: = bank shard.T
    fbt_d = nc.dram_tensor("fbt", [D, B + NSHARD], _FP8,
                           kind="ExternalInput").ap()
    # s=0/1: accV h0/h1
    acc_out = nc.dram_tensor("acc_out", [128, 2, 2, 512], _BF16,
                             kind="ExternalOutput").ap()
    # raw bf16 half-tiles shipped for host-side folding
    raw_out = nc.dram_tensor("raw_out", [128, max(NA2, 1), 2, 512], _BF16,
                             kind="ExternalOutput").ap()

    paths = _assign_paths()
    NT = len(WIDTHS)

    with tile.TileContext(nc) as tc, ExitStack() as ctx:
        const = ctx.enter_context(tc.tile_pool(name="const", bufs=1))
        fbt_pool = ctx.enter_context(tc.tile_pool(name="fbt", bufs=6))
        dp_pool = ctx.enter_context(tc.tile_pool(name="dp", bufs=4, space="PSUM"))
        tmp_pool = ctx.enter_context(tc.tile_pool(name="tmp", bufs=6))
        # A2 tmp tiles live until their DMA drains them
        tm2_pool = ctx.enter_context(tc.tile_pool(name="tm2", bufs=6))

        # PE warm-up: zero matmuls run during the first DMA so the p-state
        # ramp (3us) completes before the first real matmul. The warmup
        # PSUM tile is handed to the first half so no pool buffer is lost.
        wu_sb = const.tile([128, 512], _F32)
        nc.gpsimd.memset(wu_sb[:], 0.0)
        wu_ps = dp_pool.tile([128, 2, 512], _F32, tag="dp")
        wu_r = wu_sb[:].bitcast(_FP8).rearrange("p (c j) -> p c j", c=4)
        for _ in range(N_WARMUP):
            nc.tensor.matmul(wu_ps[:, 0], lhsT=wu_r[:, 0:2, :128], rhs=wu_r[:, 0:2],
                             start=True, stop=True,
                             perf_mode=mybir.MatmulPerfMode.DoubleRow)
        wu_reuse = [wu_ps]

        # fnt and tile 0 ride in one DMA (adjacent columns, one dest tile)
        head_sb = const.tile([128, 4, B + WIDTHS[0]], _FP8)
        fnt_sb = head_sb[:, :, :B]
        nc.sync.dma_start(
            head_sb[:],
            fbt_d[:, 0:B + WIDTHS[0]].rearrange("(c p) j -> p c j", p=128))

        accV = [const.tile([128, 2, 512], _BF16, name=f"accV{h}")
                for h in range(2)]
        for h in range(2):
            nc.vector.memset(accV[h][:], NEG)

        j0 = B
        ri = 0
        for t in range(NT):
            W = WIDTHS[t]
            if t == 0:
                fbt = head_sb[:, :, B:]
            elif W == 512:
                fbt_t = fbt_pool.tile([128, 4, 512], _FP8, tag="fbt")
                nc.sync.dma_start(
                    fbt_t[:],
                    fbt_d[:, j0:j0 + W].rearrange("(c p) j -> p c j", p=128),
                )
                fbt = fbt_t[:]
            else:
                # exact-width tile keeps the DMA descriptors contiguous
                fbt_t = fbt_pool.tile([128, 4, 256], _FP8, tag="fbts")
                nc.sync.dma_start(
                    fbt_t[:],
                    fbt_d[:, j0:j0 + W].rearrange("(c p) j -> p c j", p=128),
                )
                fbt = fbt_t[:]
            j0 += W

            for h in range(2):
                p = paths[2 * t + h]
                if wu_reuse:
                    dp = wu_reuse.pop()
                else:
                    dp = dp_pool.tile([128, 2, 512], _F32, tag="dp")
                for bi in range(2):
                    bc = 2 * h + bi
                    for dc in range(2):
                        nc.tensor.matmul(
                            dp[:, bi, :W],
                            lhsT=fnt_sb[:, 2 * dc:2 * dc + 2,
                                        bc * 128:(bc + 1) * 128],
                            rhs=fbt[:, 2 * dc:2 * dc + 2, :W],
                            start=(dc == 0), stop=(dc == 1),
                            perf_mode=mybir.MatmulPerfMode.DoubleRow,
                        )
                if p == "A":
                    a = accV[h][:, :, :W]
                    tmp = tmp_pool.tile([128, 2, 512], _BF16, tag="tmp")
                    nc.scalar.copy(out=tmp[:, :, :W], in_=dp[:, :, :W])
                    nc.vector.tensor_max(a, a, tmp[:, :, :W])
                elif p == "2":
                    tm2 = tm2_pool.tile([128, 2, 512], _BF16, tag="tm2")
                    nc.scalar.copy(out=tm2[:, :, :W], in_=dp[:, :, :W])
                    nc.sync.dma_start(raw_out[:, ri, :, :W], tm2[:, :, :W])
                    ri += 1
                else:
                    a = accV[h][:, :, :W]
                    nc.vector.tensor_max(a, a, dp[:, :, :W])

        # accV[h1] finalizes before accV[h0] in this pattern; ship it first
        nc.sync.dma_start(acc_out[:, 1], accV[1][:])
        nc.sync.dma_start(acc_out[:, 0], accV[0][:])

    nc.compile()
    return nc


def _get_module():
    if "nc" not in _cache:
        _cache["nc"] = _build_module()
    return _cache["nc"]


def kernel(features, predictions, fea_bank, score_bank, trg_idx):
    features = np.asarray(features, dtype=np.float32)
    predictions = np.asarray(predictions, dtype=np.float32)
    fea_bank = np.asarray(fea_bank, dtype=np.float32)
    score_bank = np.asarray(score_bank, dtype=np.float32)
    trg_idx = np.asarray(trg_idx, dtype=np.int32)

    sm = predictions - predictions.max(axis=1, keepdims=True)
    np.exp(sm, out=sm)
    sm /= sm.sum(axis=1, keepdims=True)
    nrm = np.maximum(np.sqrt((features * features).sum(axis=1, keepdims=True)),
                     EPS)
    f_norm = features / nrm

    fbp = np.zeros((NPAD, D), dtype=np.float32)
    fbp[:N] = fea_bank
    fbp[trg_idx] = f_norm
    sb = score_bank.copy()
    sb[trg_idx] = sm

    import ml_dtypes
    fp8 = ml_dtypes.float8_e4m3
    fnt_cols = (f_norm.T * FSCALE).astype(np.float32)

    nc = _get_module()
    in_maps = [
        {"fbt": np.ascontiguousarray(np.concatenate(
            [fnt_cols, fbp[c * NSHARD:(c + 1) * NSHARD].T],
            axis=1)).astype(fp8)}
        for c in range(NCORES)
    ]
    res = run_bass_kernel_spmd(nc, in_maps, core_ids=list(range(NCORES)))

    # which (i -> raw slot) the device shipped, and each half's (h, W)
    paths = _assign_paths()
    a2_halves = [i for i, p in enumerate(paths) if p == "2"]

    comb = np.empty((B, NCORES, 128), np.float32)
    for c, r in enumerate(res.results):
        acc = r["acc_out"].astype(np.float32)      # [128, 2h, 2bi, 512]
        raw = r["raw_out"].astype(np.float32)      # [128, NA2, 2bi, 512]
        m = acc.copy()
        for ri, i in enumerate(a2_halves):
            h, W = i % 2, WIDTHS[i // 2]
            np.maximum(m[:, h, :, :W], raw[:, ri, :, :W], out=m[:, h, :, :W])
        m = m.reshape(128, 2, 2, 4, 128).max(axis=3)
        # b = (2h + bi)*128 + p
        comb[:, c, :] = m.transpose(1, 2, 0, 3).reshape(B, 128)

    flat = comb.reshape(B, NCORES * 128)
    TOP = 16
    order = np.argpartition(-flat, TOP, axis=1)[:, :TOP]
    fv = np.take_along_axis(flat, order, axis=1)
    order = np.take_along_axis(order, np.argsort(-fv, axis=1), axis=1)
    top_core = order // 128
    top_comb = order % 128

    HC = 128
    NPOS = NSHARD // HC
    kk = np.arange(NPOS, dtype=np.int64)[None, None, :]
    pos_local = top_comb[:, :, None].astype(np.int64) + kk * HC
    rows = top_core[:, :, None].astype(np.int64) * NSHARD + pos_local
    vecs = fbp[rows.reshape(-1)].reshape(B, TOP, NPOS, D)
    dots = np.einsum("rktd,rd->rkt", vecs, f_norm, optimize=True)
    dots = np.where(rows < N, dots, np.float32(-np.inf))

    p2 = np.argsort(-dots, axis=2)[:, :, :2]
    v2 = np.take_along_axis(dots, p2, axis=2).reshape(B, 2 * TOP)
    i2 = np.take_along_axis(rows, p2, axis=2).reshape(B, 2 * TOP)

    reorder = np.lexsort((i2, -v2), axis=1)
    top_idx = np.take_along_axis(i2, reorder, axis=1)

    idx_near = top_idx[:, 1:K + 1]
    score_near = sb[idx_near].astype(np.float64)
    kl = score_near * (np.log(score_near) - sm[:, None, :].astype(np.float64))
    loss = kl.sum(axis=(1, 2)).mean()

    s64 = sm.astype(np.float64)
    neg_pred = (np.square(s64.sum(axis=0)).sum()
                - np.square(s64).sum()) / B

    return np.float32(loss + neg_pred)



# revision 4
# speedup vs baseline: 1.1084x; 1.1084x over previous
"""Trainium2 Bass kernel v7 for nn_AaD_MAPU (retrieval kNN shortlist).

Drain-bound design. The PE computes the full fp8 distance matrix
(512 q x 12544 cols per core) at full clock (~21us); the binding
constraint is PSUM-exit bandwidth: every dot value must leave PSUM
through Act (0.83 ns/v) or DVE (1.04 ns/v); GPSIMD has no PSUM port
(and no legal TensorTensor opcode), DMA cannot read PSUM. Two paths:

  B  : DVE tensor_reduce from PSUM -> per-128-col bucket maxes (fp32)
  A2 : Act exp(psum/16) -> fp8 tile, DMA ships it to the host.
       exp-companding gives ~bf16-class ranking precision at the top
       of the dot range in 1 byte, halving ship bandwidth.

Input stream: whole fbt (fnt + bank shard, fp8) is SBUF-resident; 13
chunked DMAs (2 tiles each) issued upfront so DMA streams back-to-back
and the fp8 ships queue behind them on the shared DMA engines.

Host: bucket shortlist (128-col bucket maxes from both paths), exact
fp32 re-rank of top bucket members, fp64 loss.
"""

from contextlib import ExitStack

import numpy as np

import concourse.bass as bass
import concourse.tile as tile
from concourse import bacc, mybir
from concourse.bass_utils import run_bass_kernel_spmd

B, D, N, C, K = 512, 512, 100000, 64, 5
EPS = 1e-12
NCORES = 8
NSHARD = 12544
NPAD = NSHARD * NCORES
FSCALE = 16.0

NT = 25                      # tiles: 24 x 512 + 1 x 256
TILE_W = [512] * 24 + [256]
N_WARMUP = 7
N_A2 = 26                    # Act exp->fp8 + DMA ship; rest DVE-reduce

_F32 = mybir.dt.float32
_FP8 = mybir.dt.float8e4

_cache = {}


def _assign_paths():
    """50 halves -> 'B' | '2'. Tile 24 halves (small) and the final full
    half forced B; Bresenham-spread otherwise."""
    n = 2 * NT
    forced_b = {2 * 24, 2 * 24 + 1, 2 * 23 + 1}
    quota = {"B": n - N_A2 - len(forced_b), "2": N_A2}
    issued = {"B": 0, "2": 0}
    out = []
    for i in range(n):
        if i in forced_b:
            out.append("B")
            continue
        cand = [p for p in ("B", "2") if issued[p] < quota[p]]
        p = min(cand, key=lambda q: (issued[q] + 0.5) / quota[q])
        issued[p] += 1
        out.append(p)
    return out


def _build_module():
    nc = bacc.Bacc("TRN2", target_bir_lowering=False, debug=False,
                   num_devices=NCORES)
    # columns 0:512 = f_norm.T * FSCALE, columns 512: = bank shard.T
    fbt_d = nc.dram_tensor("fbt", [D, B + NSHARD], _FP8,
                           kind="ExternalInput").ap()
    raw_out = nc.dram_tensor("raw_out", [128, max(N_A2, 1), 2, 512], _FP8,
                             kind="ExternalOutput").ap()
    # bred[p, t, h, bi, g]: B-path bucket maxes (fp32, units of 16*d)
    bred_out = nc.dram_tensor("bred_out", [128, NT, 2, 2, 4], _F32,
                              kind="ExternalOutput").ap()

    paths = _assign_paths()

    with tile.TileContext(nc) as tc, ExitStack() as ctx:
        const = ctx.enter_context(tc.tile_pool(name="const", bufs=1))
        dp_pool = ctx.enter_context(tc.tile_pool(name="dp", bufs=4, space="PSUM"))

        # PE warm-up during the first DMA chunk.
        wu_sb = const.tile([128, 512], _F32)
        nc.gpsimd.memset(wu_sb[:], 0.0)
        wu_ps = dp_pool.tile([128, 2, 512], _F32, tag="dp")
        wu_r = wu_sb[:].bitcast(_FP8).rearrange("p (c j) -> p c j", c=4)
        for _ in range(N_WARMUP):
            nc.tensor.matmul(wu_ps[:, 0], lhsT=wu_r[:, 0:2, :128], rhs=wu_r[:, 0:2],
                             start=True, stop=True,
                             perf_mode=mybir.MatmulPerfMode.DoubleRow)
        wu_reuse = [wu_ps]

        # SBUF-resident fbt in 13 chunks: ch0 = fnt + tile0 (1024 cols),
        # ch k = tiles 2k-1, 2k. All input DMAs issued upfront, no deps.
        chw = [1024] * 12 + [768]
        chunks = []
        j0 = 0
        for k, w in enumerate(chw):
            ch = const.tile([128, 4, w], _FP8, name=f"ch{k}")
            nc.sync.dma_start(
                ch[:], fbt_d[:, j0:j0 + w].rearrange("(c p) j -> p c j", p=128))
            chunks.append(ch)
            j0 += w

        fnt_sb = chunks[0][:, :, 0:512]

        def tile_rhs(t):
            # tile t = bank cols [512t, 512t+512) = fbt cols 512+512t ..
            j = 512 + 512 * t
            if t == 0:
                return chunks[0][:, :, 512:512 + TILE_W[0]]
            k = (t + 1) // 2
            off = j - sum(chw[:k])
            return chunks[k][:, :, off:off + TILE_W[t]]

        bred = const.tile([128, NT, 2, 2, 4], _F32, name="bred")
        tmps = [const.tile([128, 2, 512], _FP8, name=f"tmp{i}")
                for i in range(N_A2)]

        ti = 0   # tmp / raw slot index
        for t in range(NT):
            W = TILE_W[t]
            fbt = tile_rhs(t)
            for h in range(2):
                p = paths[2 * t + h]
                if wu_reuse:
                    dp = wu_reuse.pop()
                else:
                    dp = dp_pool.tile([128, 2, 512], _F32, tag="dp")
                for bi in range(2):
                    bc = 2 * h + bi
                    for dc in range(2):
                        nc.tensor.matmul(
                            dp[:, bi, :W],
                            lhsT=fnt_sb[:, 2 * dc:2 * dc + 2,
                                        bc * 128:(bc + 1) * 128],
                            rhs=fbt[:, 2 * dc:2 * dc + 2, :W],
                            start=(dc == 0), stop=(dc == 1),
                            perf_mode=mybir.MatmulPerfMode.DoubleRow,
                        )
                if p == "B":
                    g = W // 128
                    nc.vector.tensor_reduce(
                        out=bred[:, t, h, :, :g],
                        in_=dp[:, :, :W].rearrange("p a (g c) -> p a g c", c=128),
                        axis=mybir.AxisListType.X, op=mybir.AluOpType.max)
                else:
                    tm = tmps[ti]
                    nc.scalar.activation(out=tm[:, :, :W], in_=dp[:, :, :W],
                                         func=mybir.ActivationFunctionType.Exp,
                                         scale=1.0 / FSCALE)
                    nc.sync.dma_start(raw_out[:, ti, :, :W], tm[:, :, :W])
                    ti += 1

        nc.sync.dma_start(bred_out, bred[:])

    nc.compile()
    return nc


def _get_module():
    if "nc" not in _cache:
        _cache["nc"] = _build_module()
    return _cache["nc"]


def _host_tables():
    if "tables" in _cache:
        return _cache["tables"]
    paths = _assign_paths()
    fine = {0: [], 1: []}   # per h: list of (t, src, slot)
    ti = 0
    for t in range(NT):
        for h in range(2):
            if paths[2 * t + h] == "B":
                fine[h].append((t, "B", 0))
            else:
                fine[h].append((t, "2", ti))
                ti += 1
    _cache["tables"] = (paths, fine)
    return _cache["tables"]


def kernel(features, predictions, fea_bank, score_bank, trg_idx):
    features = np.asarray(features, dtype=np.float32)
    predictions = np.asarray(predictions, dtype=np.float32)
    fea_bank = np.asarray(fea_bank, dtype=np.float32)
    score_bank = np.asarray(score_bank, dtype=np.float32)
    trg_idx = np.asarray(trg_idx, dtype=np.int32)

    sm = predictions - predictions.max(axis=1, keepdims=True)
    np.exp(sm, out=sm)
    sm /= sm.sum(axis=1, keepdims=True)
    nrm = np.maximum(np.sqrt((features * features).sum(axis=1, keepdims=True)),
                     EPS)
    f_norm = features / nrm

    fbp = np.zeros((NPAD, D), dtype=np.float32)
    fbp[:N] = fea_bank
    fbp[trg_idx] = f_norm
    sb = score_bank.copy()
    sb[trg_idx] = sm

    import ml_dtypes
    fp8 = ml_dtypes.float8_e4m3
    fnt_cols = (f_norm.T * FSCALE).astype(np.float32)

    nc = _get_module()
    in_maps = [
        {"fbt": np.ascontiguousarray(np.concatenate(
            [fnt_cols, fbp[c * NSHARD:(c + 1) * NSHARD].T],
            axis=1)).astype(fp8)}
        for c in range(NCORES)
    ]
    res = run_bass_kernel_spmd(nc, in_maps, core_ids=list(range(NCORES)))

    paths, fine = _host_tables()

    breds = [r["bred_out"] for r in res.results]            # 16*d fp32
    raws = [r["raw_out"].astype(np.float32) for r in res.results]  # exp(d) fp8

    TOPF = 16   # buckets kept per query

    # per h: fine values [128p, 2bi, NCORES, nf, 4] in d-units
    rows_h = [None, None]
    for h in range(2):
        fl = fine[h]
        nf = len(fl)
        fv = np.full((128, 2, NCORES, nf, 4), -np.inf, np.float32)
        with np.errstate(divide="ignore"):
            for c in range(NCORES):
                for fi, (t, src, slot) in enumerate(fl):
                    g = TILE_W[t] // 128
                    if src == "B":
                        fv[:, :, c, fi, :g] = breds[c][:, t, h, :, :g] / FSCALE
                    else:
                        v = raws[c][:, slot, :, :TILE_W[t]]
                        v = v.reshape(128, 2, g, 128).max(axis=3)
                        fv[:, :, c, fi, :g] = np.log(v)
        fbase = np.empty((NCORES, nf, 4), np.int64)
        for c in range(NCORES):
            for fi, (t, src, slot) in enumerate(fl):
                for g in range(4):
                    fbase[c, fi, g] = c * NSHARD + 512 * t + 128 * g
        fvf = fv.reshape(128, 2, NCORES * nf * 4)
        fbf = fbase.reshape(NCORES * nf * 4)
        selF = np.argpartition(-fvf, TOPF, axis=2)[:, :, :TOPF]
        rows_h[h] = (fbf[selF][..., None] + np.arange(128, dtype=np.int64)
                     ).reshape(128, 2, TOPF * 128)

    ncand = TOPF * 128
    rows_all = np.zeros((B, ncand), np.int64)
    for h in range(2):
        for bi in range(2):
            q0 = (2 * h + bi) * 128
            rows_all[q0:q0 + 128] = rows_h[h][:, bi]

    # ---- exact re-rank -----------------------------------------------------
    dots = np.empty((B, ncand), np.float32)
    CH = 64
    for q0 in range(0, B, CH):
        rr = rows_all[q0:q0 + CH]
        vec = fbp[rr.reshape(-1)].reshape(CH, ncand, D)
        dots[q0:q0 + CH] = np.einsum("qkd,qd->qk", vec,
                                     f_norm[q0:q0 + CH], optimize=True)
    dots = np.where(rows_all < N, dots, np.float32(-np.inf))

    # top-6 rows, ties by lower row id (match jax top_k); buckets are
    # disjoint so no dedupe needed.
    order = np.lexsort((rows_all, -dots), axis=1)[:, :K + 1]
    top_idx = np.take_along_axis(rows_all, order, axis=1)

    idx_near = top_idx[:, 1:K + 1]
    score_near = sb[idx_near].astype(np.float64)
    kl = score_near * (np.log(score_near) - sm[:, None, :].astype(np.float64))
    loss = kl.sum(axis=(1, 2)).mean()

    s64 = sm.astype(np.float64)
    neg_pred = (np.square(s64.sum(axis=0)).sum()
                - np.square(s64).sum()) / B

    return np.float32(loss + neg_pred)
